# revision 1
# baseline (speedup 1.0000x reference)
"""NeuralTPP (GRU + monotone hazard MLP loglik) Bass kernel for 8 trn2 cores.

Problem: B=4096 samples, L=512 steps. Per step t:
  hazard:  pre = tau*w1_tau + h@w1_h.T + b1 ; a = tanh(pre)
           raw = a@w2 + b2 ; phi = softplus(raw)
           dphi = sigmoid(raw) * ((1-a^2)*w1_tau)@w2 ; lam = softplus(dphi)+eps
           tot += sum((log(lam) - phi) * m)
  GRU:     r,z,n gates with scalar input tau; h' = h + m*(1-z)*(n-h)
Output: tot / (sum(mask) + eps)   (scalar f32)

Sharding: pure data parallel, batch split 8 x 512. Each core runs the full
512-step scan on its 512 samples in H-major layout [gate-dim, batch]:
  - MM_G: one K=35 matmul/step -> PSUM rows [z_neg; r_pre; m_bcast; h_n]
          (z pre-acts negated so sigmoid gives zc = 1-z directly)
  - MM_N: K=3 matmul -> i_n (tau-only GRU n-gate input)
  - MM_P: K=35 matmul -> hazard pre for step t into row-block t%4 of a
          [128,512] PSUM bank (4 steps share a bank)
  - ACT sigmoid [64,512] -> [zc; r]; DVE [q;rh] = [zc;r] * [m_b;h_n];
    t2 = rh + i_n; ACT tanh -> n; d = n-h; f = q*d; h' = h+f
  - every 4 steps: tanh/square on the [128,512] pre bank, two K=128
    dot matmuls -> raw/s rows, copied into SBUF step-stacked tiles
  - end: batched loglik tail over [128,512] tiles (sigmoid/softplus/ln),
    per-partition sums via scalar_tensor_tensor accum_out
Host: sums the 8 cores' [128,4] partials in f64, divides by mask sum.
"""

import numpy as np

B, L, H, HH = 4096, 512, 32, 32
EPS = 1e-8
NCORES = 8
BC = B // NCORES  # 512 samples per core

_CACHE = {}


def _build_module():
    import concourse.bacc as bacc
    import concourse.mybir as mybir
    import concourse.tile as tile

    f32 = mybir.dt.float32
    AF = mybir.ActivationFunctionType
    ALU = mybir.AluOpType

    nc = bacc.Bacc()

    xr_d = nc.dram_tensor("xr", [L, 3, BC], f32, kind="ExternalInput")
    # packed consts: w1c = [lhsG | lhsN | lhsP] on 35 partitions,
    # w2c = [lhsR | lhsS | scal(c0,b2,eps)] on 128 partitions
    w1c_d = nc.dram_tensor("w1c", [35, 192], f32, kind="ExternalInput")
    w2c_d = nc.dram_tensor("w2c", [128, 11], f32, kind="ExternalInput")
    hx0_d = nc.dram_tensor("hx0", [35, BC], f32, kind="ExternalInput")
    acc_d = nc.dram_tensor("acc_out", [128, 4], f32, kind="ExternalOutput")

    with tile.TileContext(nc) as tc:
        with (
            tc.tile_pool(name="consts", bufs=1) as consts,
            tc.tile_pool(name="hx", bufs=3) as hx_pool,
            tc.tile_pool(name="work", bufs=3) as work,
            tc.tile_pool(name="grp", bufs=2) as grp,
            tc.tile_pool(name="store", bufs=1) as store,
            tc.tile_pool(name="tail", bufs=2) as tailp,
            tc.tile_pool(name="gP", bufs=2, space="PSUM") as gP,
            tc.tile_pool(name="nP", bufs=2, space="PSUM") as nP,
            tc.tile_pool(name="pP", bufs=2, space="PSUM") as pP,
            tc.tile_pool(name="dP", bufs=2, space="PSUM") as dP,
        ):
            w1c = consts.tile([35, 192], f32)
            w2c = consts.tile([128, 11], f32)
            nc.sync.dma_start(w1c[:], w1c_d[:])
            nc.sync.dma_start(w2c[:], w2c_d[:])
            lhsG, lhsN, lhsP = w1c[:, 0:128], w1c[:, 128:160], w1c[:, 160:192]
            lhsR, lhsS, c0b = w2c[:, 0:4], w2c[:, 4:8], w2c[:, 8:11]

            # raw / s values for all 512 steps, stacked 128 steps per column
            # block: value for step t lives at [t % 128, (t // 128)*512 + b]
            RAWa = store.tile([128, 4, BC], f32, tag="rawa")
            Sa = store.tile([128, 4, BC], f32, tag="sa")
            ACC = store.tile([128, 4], f32, tag="accs")

            # h carry + per-step (tau, m, 1) rows; rows: 0:32 h, 32 tau,
            # 33 m, 34 ones
            hx = hx_pool.tile([35, BC], f32, tag="hx")
            nc.sync.dma_start(hx[:], hx0_d[:])

            pbank = None
            for t in range(L):
                if t % 4 == 0:
                    pbank = pP.tile([128, BC], f32, tag="pbank")
                # hazard pre-activation for this step's h (pre-update)
                nc.tensor.matmul(
                    pbank[32 * (t % 4) : 32 * (t % 4) + 32, :],
                    lhsP,
                    hx[0:35, :],
                    start=True,
                    stop=True,
                    tile_position=(0, 32 * (t % 4)),
                )

                if t < L - 1:
                    gbank = gP.tile([128, BC], f32, tag="gbank")
                    nbank = nP.tile([32, BC], f32, tag="nbank")
                    nc.tensor.matmul(
                        gbank[:], lhsG, hx[0:35, :], start=True, stop=True
                    )
                    nc.tensor.matmul(
                        nbank[:], lhsN, hx[0:35, :], start=True, stop=True
                    )

                    S = work.tile([64, BC], f32, tag="S")
                    nc.scalar.activation(S[:], gbank[0:64, :], AF.Sigmoid)
                    U = work.tile([64, BC], f32, tag="U")
                    nc.vector.tensor_mul(U[:], S[:], gbank[64:128, :])
                    T2 = work.tile([32, BC], f32, tag="T2")
                    nc.vector.tensor_add(T2[:], U[32:64, :], nbank[:])
                    N_ = work.tile([32, BC], f32, tag="N")
                    nc.scalar.activation(N_[:], T2[:], AF.Tanh)
                    D = work.tile([32, BC], f32, tag="D")
                    nc.vector.tensor_sub(D[:], N_[:], hx[0:32, :])
                    F = work.tile([32, BC], f32, tag="F")
                    nc.vector.tensor_mul(F[:], U[0:32, :], D[:])

                    hx_next = hx_pool.tile([35, BC], f32, tag="hx")
                    nc.vector.tensor_add(hx_next[0:32, :], hx[0:32, :], F[:])
                    nc.sync.dma_start(hx_next[32:35, :], xr_d[t + 1])
                    hx = hx_next

                if t % 4 == 3:
                    g = t // 4
                    A4 = grp.tile([128, BC], f32, tag="A4")
                    nc.scalar.activation(A4[:], pbank[:], AF.Tanh)
                    SQ4 = grp.tile([128, BC], f32, tag="SQ4")
                    nc.scalar.activation(SQ4[:], A4[:], AF.Square)
                    dbank = dP.tile([64, BC], f32, tag="dbank")
                    nc.tensor.matmul(
                        dbank[0:4, :], lhsR, A4[:], start=True, stop=True,
                        tile_position=(0, 0),
                    )
                    nc.tensor.matmul(
                        dbank[32:36, :], lhsS, SQ4[:], start=True, stop=True,
                        tile_position=(0, 32),
                    )
                    blk, row = g // 32, 4 * (g % 32)
                    stR = grp.tile([4, BC], f32, tag="stR", name="stR")
                    stS = grp.tile([4, BC], f32, tag="stS", name="stS")
                    nc.scalar.activation(stR[:], dbank[0:4, :], AF.Copy)
                    nc.scalar.activation(stS[:], dbank[32:36, :], AF.Copy)
                    nc.sync.dma_start(RAWa[row : row + 4, blk, :], stR[:])
                    nc.sync.dma_start(Sa[row : row + 4, blk, :], stS[:])

            # ---- batched loglik tail ----

            Mb, SG, ND, PH, SPD, LGL, LL, LLM = ([None] * 4 for _ in range(8))
            for i in range(4):
                Mb[i] = tailp.tile([128, BC], f32, tag="Mb", name=f"Mb{i}")
                nc.sync.dma_start(Mb[i][:], xr_d[128 * i : 128 * (i + 1), 1, :])
            for i in range(4):
                SG[i] = tailp.tile([128, BC], f32, tag="SG", name=f"SG{i}")
                nc.scalar.activation(
                    SG[i][:], RAWa[:, i, :], AF.Sigmoid, bias=c0b[:, 1:2]
                )
            for i in range(4):
                ND[i] = tailp.tile([128, BC], f32, tag="ND", name=f"ND{i}")
                nc.vector.scalar_tensor_tensor(
                    ND[i][:], Sa[:, i, :], c0b[:, 0:1], SG[i][:],
                    op0=ALU.subtract, op1=ALU.mult,
                )
            # softplus(x) = ln(1 + exp(x)) — this walrus act table set has no
            # native softplus; exp and ln share natural_log_exp_and_others.
            # Ranges are small (|raw|, |dphi| < ~8) so exp cannot overflow.
            for i in range(4):
                EX = tailp.tile([128, BC], f32, tag="EX", name=f"EX{i}")
                nc.scalar.activation(EX[:], RAWa[:, i, :], AF.Exp, bias=c0b[:, 1:2])
                PH[i] = tailp.tile([128, BC], f32, tag="PH", name=f"PH{i}")
                nc.scalar.activation(PH[i][:], EX[:], AF.Ln, bias=1.0)
                EX2 = tailp.tile([128, BC], f32, tag="EX2", name=f"EX2{i}")
                nc.scalar.activation(EX2[:], ND[i][:], AF.Exp, scale=-1.0)
                SPD[i] = tailp.tile([128, BC], f32, tag="SPD", name=f"SPD{i}")
                nc.scalar.activation(SPD[i][:], EX2[:], AF.Ln, bias=1.0)
            for i in range(4):
                LGL[i] = tailp.tile([128, BC], f32, tag="LGL", name=f"LGL{i}")
                nc.scalar.activation(LGL[i][:], SPD[i][:], AF.Ln, bias=c0b[:, 2:3])
            for i in range(4):
                LL[i] = tailp.tile([128, BC], f32, tag="LL", name=f"LL{i}")
                nc.vector.tensor_sub(LL[i][:], LGL[i][:], PH[i][:])
                LLM[i] = tailp.tile([128, BC], f32, tag="LLM", name=f"LLM{i}")
                nc.vector.scalar_tensor_tensor(
                    LLM[i][:], LL[i][:], 0.0, Mb[i][:],
                    op0=ALU.add, op1=ALU.mult,
                    accum_out=ACC[:, i : i + 1],
                )
            nc.sync.dma_start(acc_d[:], ACC[:])

    nc.finalize()
    return nc


def _prep_host(inputs):
    d = {k: np.asarray(v, np.float32) for k, v in inputs.items()}
    w_ih, w_hh = d["w_ih"], d["w_hh"]
    b_ih, b_hh = d["b_ih"], d["b_hh"]
    w1, b1, w2, b2 = d["w1"], d["b1"], d["w2"], d["b2"]
    w1_tau, w1_h = w1[:, 0], w1[:, 1:]

    lhsG = np.zeros((35, 128), np.float32)
    # z_neg block (cols 0:32): gives sigmoid -> 1-z
    lhsG[0:32, 0:32] = -w_hh[32:64, :].T
    lhsG[32, 0:32] = -w_ih[32:64, 0]
    lhsG[34, 0:32] = -(b_ih[32:64] + b_hh[32:64])
    # r block
    lhsG[0:32, 32:64] = w_hh[0:32, :].T
    lhsG[32, 32:64] = w_ih[0:32, 0]
    lhsG[34, 32:64] = b_ih[0:32] + b_hh[0:32]
    # mask broadcast block
    lhsG[33, 64:96] = 1.0
    # h_n block (recurrent part of n gate, with b_hh only)
    lhsG[0:32, 96:128] = w_hh[64:96, :].T
    lhsG[34, 96:128] = b_hh[64:96]

    lhsN = np.zeros((35, 32), np.float32)
    lhsN[32, :] = w_ih[64:96, 0]
    lhsN[34, :] = b_ih[64:96]

    lhsP = np.zeros((35, 32), np.float32)
    lhsP[0:32, :] = w1_h.T
    lhsP[32, :] = w1_tau
    lhsP[34, :] = b1

    c = w1_tau * w2
    lhsR = np.zeros((128, 4), np.float32)
    lhsS = np.zeros((128, 4), np.float32)
    for g in range(4):
        lhsR[32 * g : 32 * g + 32, g] = w2
        lhsS[32 * g : 32 * g + 32, g] = c
    scal = np.tile(np.array([[c.sum(), b2[0], EPS]], np.float32), (128, 1))
    w1c = np.concatenate([lhsG, lhsN, lhsP], axis=1)  # [35, 192]
    w2c = np.concatenate([lhsR, lhsS, scal], axis=1)  # [128, 11]

    deltas, mask = d["deltas"], d["mask"]
    in_maps = []
    for i in range(NCORES):
        sl = slice(i * BC, (i + 1) * BC)
        xr = np.empty((L, 3, BC), np.float32)
        xr[:, 0, :] = deltas[sl].T
        xr[:, 1, :] = mask[sl].T
        xr[:, 2, :] = 1.0
        hx0 = np.zeros((35, BC), np.float32)
        hx0[32:35, :] = xr[0]
        in_maps.append({"xr": xr, "w1c": w1c, "w2c": w2c, "hx0": hx0})
    return in_maps


def run_on_device(inputs, trace=False):
    from concourse.bass_utils import run_bass_kernel_spmd

    if "nc" not in _CACHE:
        _CACHE["nc"] = _build_module()
    nc = _CACHE["nc"]
    in_maps = _prep_host(inputs)
    res = run_bass_kernel_spmd(nc, in_maps, core_ids=list(range(NCORES)), trace=trace)
    tot = 0.0
    for r in res.results:
        tot += np.asarray(r["acc_out"], np.float64).sum()
    msum = np.asarray(inputs["mask"], np.float64).sum()
    out = np.float32(tot / (msum + EPS))
    return np.asarray(out, np.float32), res


def kernel(**inputs):
    out, _ = run_on_device(inputs, trace=False)
    return out



# revision 2
# speedup vs baseline: 70.1189x; 70.1189x over previous
"""NeuralTPP Bass kernel v2 — 8 trn2 cores, data-parallel batch shard.

Per core: 512 samples x 512 steps as TWO phase-shifted halves of 256 cols
(two independent dependency chains keep PE/ACT/DVE busy). Emission is
phased (pre-tanh ops for both halves, then post-tanh ops) to avoid
head-of-line blocking in the engine queues.

hx [67,HC] bf16: rows 0:32 h_{t-1}, 32:64 F_{t-1}, 64 tau, 65 kill, 66 one.
lhs G/P duplicate the h-weights on the F rows, so matmuls see
h_t = h_{t-1}+F_{t-1} without materializing it (hadd runs off-path).

Per half h, per step t:
  PE   MM_G -> bank[0:96,0:HC] = [zc_neg+kill | r | h_n]
  PE   MM_I -> bank[0:32,HC:]  = i_n (start, accumulated)
  PE   MM_P -> bank[32:64,HC:] = hazard pre
  ACT  sigma(bank[0:64,0:HC]) -> SR=[q;r] bf16   (q=(1-z)*m via kill row)
  DVE  hadd: hx_next[0:32] = hx[0:32]+hx[32:64]  (= h_t, off-path)
  DVE  rh = r*h_n (mixed psum read) -> bf16
  PE   MM_ACC: identity@rh accumulated onto i_n -> T2 (stop)
  ACT  tanh(bank[0:64,HC:]) -> Acoll rows 64*(t%2) = [n;a] bf16
  DVE  D = n - h_t ; F = q*D -> hx_next[32:64] ; sq = a*a -> SQcoll
  PE   raw-dot per 2 steps / s-dot per 4 steps (phase-accumulated K=128
       dots into [32,HC] blocks; row order works out to row = t%128),
       emitted one step late to avoid PE head-of-line blocking.
Every 128 steps rawbank/sbank drain via ACT copy to SBUF f32 slabs; the
batched loglik tail runs at the end (2 act-table switches total), masked
accumulation via scalar_tensor_tensor accum_out. Host sums [128,2] f64.
"""

import numpy as np

B, L, H, HH = 4096, 512, 32, 32
EPS = 1e-8
NCORES = 8
BC = B // NCORES       # 512 samples per core
HC = BC // 2           # 256 per half
KILL = -40.0

_CACHE = {}


def _build_module():
    import concourse.bacc as bacc
    import concourse.mybir as mybir
    import concourse.tile as tile

    f32 = mybir.dt.float32
    bf16 = mybir.dt.bfloat16
    AF = mybir.ActivationFunctionType
    ALU = mybir.AluOpType

    nc = bacc.Bacc()

    # xr rows: 0 tau, 1 kill(-40*(1-m)), 2 ones   (bf16, step-major)
    xr_d = nc.dram_tensor("xr", [L, 3, BC], bf16, kind="ExternalInput")
    # lhsg cols: [G(96) | I(32) | P(32) | ident(32)]
    lhsg_d = nc.dram_tensor("lhsg", [67, 192], bf16, kind="ExternalInput")
    # lhsd: 32 R-phase blocks [64,32] then 8 S-phase blocks [128,32]
    lhsd_d = nc.dram_tensor("lhsd", [128, 1280], bf16, kind="ExternalInput")
    cst_d = nc.dram_tensor("cst", [128, 3], f32, kind="ExternalInput")
    hx0_d = nc.dram_tensor("hx0", [67, BC], bf16, kind="ExternalInput")
    # mask in tail layout: [half, round, step, col]
    mt_d = nc.dram_tensor("mt", [2, 4, 128, HC], f32, kind="ExternalInput")
    acc_d = nc.dram_tensor("acc_out", [128, 2], f32, kind="ExternalOutput")

    NR = 4  # drain rounds per half (512 steps / 128)

    with tile.TileContext(nc) as tc:
        with (
            tc.tile_pool(name="consts", bufs=1) as consts,
            tc.tile_pool(name="hx", bufs=3) as hx_pool,
            tc.tile_pool(name="work", bufs=3) as work,
            tc.tile_pool(name="coll", bufs=2) as coll,
            tc.tile_pool(name="slab", bufs=1) as slabp,
            tc.tile_pool(name="tail", bufs=2) as tailp,
            tc.tile_pool(name="gP", bufs=2, space="PSUM") as gP,
            tc.tile_pool(name="rP", bufs=1, space="PSUM") as rP,
            tc.tile_pool(name="sP", bufs=1, space="PSUM") as sP,
        ):
            lhsg = consts.tile([67, 192], bf16)
            lhsd = consts.tile([128, 1280], bf16)
            cst = consts.tile([128, 3], f32)
            nc.sync.dma_start(lhsg[:], lhsg_d[:])
            nc.sync.dma_start(lhsd[:], lhsd_d[:])
            nc.sync.dma_start(cst[:], cst_d[:])
            lhsG, lhsI = lhsg[:, 0:96], lhsg[:, 96:128]
            lhsP, lhsE = lhsg[:, 128:160], lhsg[0:32, 160:192]
            lhsRp = [lhsd[0:64, 32 * j : 32 * j + 32] for j in range(32)]
            lhsSp = [
                lhsd[:, 1024 + 32 * j : 1024 + 32 * j + 32] for j in range(8)
            ]

            # raw/s slabs: [128, NR, HC] f32 per half
            RAW = [slabp.tile([128, NR, HC], f32, name=f"RAW{h}") for h in range(2)]
            SV = [slabp.tile([128, NR, HC], f32, name=f"SV{h}") for h in range(2)]
            ACC = slabp.tile([128, 2], f32, name="ACCS")

            hx = [None, None]
            for h in range(2):
                t0 = hx_pool.tile([67, HC], bf16, tag=f"hx{h}", name=f"hx0_{h}")
                nc.sync.dma_start(t0[:], hx0_d[:, h * HC : (h + 1) * HC])
                hx[h] = t0

            st = [
                dict(A=None, SQ=None, SR=None, bank=None, hxn=None, Fc=None,
                     dots=[])
                for _ in range(2)
            ]
            for h in range(2):
                Fc0 = work.tile([32, HC], bf16, tag=f"Fc{h}", name=f"Fc0_{h}")
                nc.vector.memset(Fc0[:], 0.0)
                st[h]["Fc"] = Fc0
            rawb = sb = None

            for t in range(L):
                if t % 128 == 0:
                    rawb = rP.tile([128, BC], f32, tag="rawb", name=f"rawb{t}")
                    sb = sP.tile([128, BC], f32, tag="sb", name=f"sb{t}")
                last = t == L - 1

                # ---- phase PRE: matmuls + sigma + hadd ----
                for h in range(2):
                    s = st[h]
                    s["A"] = coll.tile(
                        [64, HC], bf16, tag=f"A{h}", name=f"A{h}_{t}"
                    )
                    if t % 4 == 0:
                        s["SQ"] = coll.tile(
                            [128, HC], bf16, tag=f"SQ{h}", name=f"SQ{h}_{t}"
                        )
                    bank = gP.tile(
                        [128, 2 * HC], f32, tag=f"G{h}", name=f"G{h}_{t}"
                    )
                    s["bank"] = bank
                    nc.tensor.matmul(
                        bank[0:96, 0:HC], lhsG, hx[h][:], start=True, stop=True
                    )
                    nc.tensor.matmul(
                        bank[0:32, HC : 2 * HC], lhsI, hx[h][:],
                        start=True, stop=False, skip_group_check=True,
                    )
                    nc.tensor.matmul(
                        bank[32:64, HC : 2 * HC], lhsP, hx[h][:],
                        start=True, stop=True, tile_position=(0, 32),
                        skip_group_check=True,
                    )
                    SR = work.tile([64, HC], bf16, tag=f"SR{h}")
                    s["SR"] = SR
                    nc.scalar.activation(SR[:], bank[0:64, 0:HC], AF.Sigmoid)
                    if not last:
                        hxn = hx_pool.tile([67, HC], bf16, tag=f"hx{h}")
                        s["hxn"] = hxn
                        nc.vector.tensor_add(
                            hxn[0:32, :], hx[h][0:32, :], s["Fc"]
                        )
                        nc.sync.dma_start(
                            hxn[64:67, :], xr_d[t + 1, :, h * HC : (h + 1) * HC]
                        )

                # rh + T2 accumulation
                for h in range(2):
                    s = st[h]
                    rh = work.tile([32, HC], bf16, tag=f"rh{h}")
                    nc.vector.tensor_mul(
                        rh[:], s["SR"][32:64, :], s["bank"][64:96, 0:HC]
                    )
                    nc.tensor.matmul(
                        s["bank"][0:32, HC : 2 * HC], lhsE, rh[:],
                        start=False, stop=True, skip_group_check=True,
                    )

                # deferred dots from the previous step (PE queue: behind MMs)
                for h in range(2):
                    for args in st[h]["dots"]:
                        nc.tensor.matmul(*args[0], **args[1])
                    st[h]["dots"] = []

                # ---- phase POST: tanh + h-update + collections ----
                for h in range(2):
                    s = st[h]
                    nc.scalar.activation(
                        s["A"][:], s["bank"][0:64, HC : 2 * HC], AF.Tanh
                    )

                for h in range(2):
                    s = st[h]
                    if not last:
                        D = work.tile([32, HC], bf16, tag=f"D{h}")
                        nc.vector.tensor_sub(
                            D[:], s["A"][0:32, :], s["hxn"][0:32, :]
                        )
                        nc.vector.tensor_mul(
                            s["hxn"][32:64, :], s["SR"][0:32, :], D[:]
                        )
                        Fc = work.tile([32, HC], bf16, tag=f"Fc{h}")
                        nc.vector.tensor_copy(Fc[:], s["hxn"][32:64, :])
                        s["Fc"] = Fc
                        hx[h] = s["hxn"]

                for h in range(2):
                    s = st[h]
                    cs = slice(h * HC, (h + 1) * HC)
                    A, SQ = s["A"], s["SQ"]
                    sr = 32 * (t % 4)
                    nc.vector.tensor_mul(
                        SQ[sr : sr + 32, :], A[32:64, :], A[32:64, :]
                    )

                    tr = t % 128
                    if True:
                        blk, ph = 32 * (tr // 32), tr % 32
                        s["dots"].append((
                            (rawb[blk : blk + 32, cs], lhsRp[ph], A[:]),
                            dict(start=(ph == 0), stop=(ph == 31),
                                 tile_position=(0, blk),
                                 skip_group_check=True),
                        ))
                    if t % 4 == 3:
                        blk, ph = 32 * (tr // 32), (tr % 32) // 4
                        s["dots"].append((
                            (sb[blk : blk + 32, cs], lhsSp[ph], SQ[:]),
                            dict(start=(ph == 0), stop=(ph == 7),
                                 tile_position=(0, blk),
                                 skip_group_check=True),
                        ))
                    if t % 128 == 127 or last:
                        # flush dots before draining on the last round
                        for args in s["dots"]:
                            nc.tensor.matmul(*args[0], **args[1])
                        s["dots"] = []
                    if t % 128 == 127:
                        rnd = t // 128
                        nc.scalar.activation(
                            RAW[h][:, rnd, :], rawb[:, cs], AF.Copy
                        )
                        nc.scalar.activation(
                            SV[h][:, rnd, :], sb[:, cs], AF.Copy
                        )

            # ---- batched loglik tail ----
            b2ap, c0ap, epap = cst[:, 0:1], cst[:, 1:2], cst[:, 2:3]
            MT, SG, ND = [None, None], [None, None], [None, None]
            for h in range(2):
                MT[h] = tailp.tile([128, NR, HC], f32, tag="MT", name=f"MT{h}")
                nc.sync.dma_start(MT[h][:], mt_d[h])
                SG[h] = tailp.tile([128, NR, HC], f32, tag="SG", name=f"SG{h}")
                nc.scalar.activation(
                    SG[h][:], RAW[h][:], AF.Sigmoid, bias=b2ap
                )
                ND[h] = tailp.tile([128, NR, HC], f32, tag="ND", name=f"ND{h}")
                nc.vector.scalar_tensor_tensor(
                    ND[h][:], SV[h][:], c0ap, SG[h][:],
                    op0=ALU.subtract, op1=ALU.mult,
                )
            for h in range(2):
                EX = tailp.tile([128, NR, HC], f32, tag="EX", name=f"EX{h}")
                nc.scalar.activation(EX[:], RAW[h][:], AF.Exp, bias=b2ap)
                PH = tailp.tile([128, NR, HC], f32, tag="PH", name=f"PH{h}")
                nc.scalar.activation(PH[:], EX[:], AF.Ln, bias=1.0)
                EX2 = tailp.tile([128, NR, HC], f32, tag="EX2", name=f"EX2{h}")
                nc.scalar.activation(EX2[:], ND[h][:], AF.Exp, scale=-1.0)
                SPD = tailp.tile([128, NR, HC], f32, tag="SPD", name=f"SPD{h}")
                nc.scalar.activation(SPD[:], EX2[:], AF.Ln, bias=1.0)
                LGL = tailp.tile([128, NR, HC], f32, tag="LGL", name=f"LGL{h}")
                nc.scalar.activation(LGL[:], SPD[:], AF.Ln, bias=epap)
                LLd = tailp.tile([128, NR, HC], f32, tag="LL", name=f"LL{h}")
                nc.vector.tensor_sub(LLd[:], LGL[:], PH[:])
                LLM = tailp.tile([128, NR, HC], f32, tag="LLM", name=f"LLM{h}")
                nc.vector.scalar_tensor_tensor(
                    LLM[:], LLd[:], 0.0, MT[h][:],
                    op0=ALU.add, op1=ALU.mult,
                    accum_out=ACC[:, h : h + 1],
                )
            nc.sync.dma_start(acc_d[:], ACC[:])

    nc.finalize()
    return nc


def _prep_host(inputs):
    import ml_dtypes

    bf = ml_dtypes.bfloat16
    d = {k: np.asarray(v, np.float32) for k, v in inputs.items()}
    w_ih, w_hh = d["w_ih"], d["w_hh"]
    b_ih, b_hh = d["b_ih"], d["b_hh"]
    w1, b1, w2, b2 = d["w1"], d["b1"], d["w2"], d["b2"]
    w1_tau, w1_h = w1[:, 0], w1[:, 1:]

    # G blocks: [zc_neg(0:32) | r(32:64) | h_n(64:96)], h-weights
    # duplicated on F rows 32:64
    lhsG = np.zeros((67, 96), np.float32)
    hw = np.zeros((32, 96), np.float32)
    hw[:, 0:32] = -w_hh[32:64, :].T
    hw[:, 32:64] = w_hh[0:32, :].T
    hw[:, 64:96] = w_hh[64:96, :].T
    lhsG[0:32] = hw
    lhsG[32:64] = hw
    lhsG[64, 0:32] = -w_ih[32:64, 0]
    lhsG[64, 32:64] = w_ih[0:32, 0]
    lhsG[65, 0:32] = 1.0                       # kill row -> zc block
    lhsG[66, 0:32] = -(b_ih[32:64] + b_hh[32:64])
    lhsG[66, 32:64] = b_ih[0:32] + b_hh[0:32]
    lhsG[66, 64:96] = b_hh[64:96]

    lhsI = np.zeros((67, 32), np.float32)
    lhsI[64, :] = w_ih[64:96, 0]
    lhsI[66, :] = b_ih[64:96]

    lhsP = np.zeros((67, 32), np.float32)
    lhsP[0:32, :] = w1_h.T
    lhsP[32:64, :] = w1_h.T
    lhsP[64, :] = w1_tau
    lhsP[66, :] = b1

    lhsg = np.zeros((67, 192), np.float32)
    lhsg[:, 0:96] = lhsG
    lhsg[:, 96:128] = lhsI
    lhsg[:, 128:160] = lhsP
    lhsg[0:32, 160:192] = np.eye(32, dtype=np.float32)

    c = w1_tau * w2
    lhsd = np.zeros((128, 1280), np.float32)
    for j in range(32):  # raw-dot phase j: out row j, K=64, a at rows 32:64
        lhsd[32:64, 32 * j + j] = w2
    for j in range(8):
        for g in range(4):
            lhsd[32 * g : 32 * g + 32, 1024 + 32 * j + 4 * j + g] = c

    cst = np.zeros((128, 3), np.float32)
    cst[:, 0] = b2[0]
    cst[:, 1] = c.sum()
    cst[:, 2] = EPS

    deltas, mask = d["deltas"], d["mask"]
    in_maps = []
    for i in range(NCORES):
        sl = slice(i * BC, (i + 1) * BC)
        dT = deltas[sl].T          # [L, BC]
        mT = mask[sl].T            # [L, BC]
        xr = np.empty((L, 3, BC), np.float32)
        xr[:, 0, :] = dT
        xr[:, 1, :] = KILL * (1.0 - mT)
        xr[:, 2, :] = 1.0
        hx0 = np.zeros((67, BC), np.float32)
        hx0[64:67, :] = xr[0]
        mt = np.empty((2, 4, 128, HC), np.float32)
        for h in range(2):
            for r in range(4):
                mt[h, r] = mT[128 * r : 128 * (r + 1), h * HC : (h + 1) * HC]
        in_maps.append(
            {
                "xr": xr.astype(bf),
                "lhsg": lhsg.astype(bf),
                "lhsd": lhsd.astype(bf),
                "cst": cst,
                "hx0": hx0.astype(bf),
                "mt": mt,
            }
        )
    return in_maps


def run_on_device(inputs, trace=False):
    from concourse.bass_utils import run_bass_kernel_spmd

    if "nc" not in _CACHE:
        _CACHE["nc"] = _build_module()
    nc = _CACHE["nc"]
    in_maps = _prep_host(inputs)
    res = run_bass_kernel_spmd(nc, in_maps, core_ids=list(range(NCORES)), trace=trace)
    tot = 0.0
    for r in res.results:
        tot += np.asarray(r["acc_out"], np.float64).sum()
    msum = np.asarray(inputs["mask"], np.float64).sum()
    out = np.float32(tot / (msum + EPS))
    return np.asarray(out, np.float32), res


def kernel(**inputs):
    out, _ = run_on_device(inputs, trace=False)
    return out


# revision 3
# speedup vs baseline: 78.6547x; 1.1217x over previous
"""NeuralTPP Bass kernel v2 — 8 trn2 cores, data-parallel batch shard.

Per core: 512 samples x 512 steps as TWO phase-shifted halves of 256 cols
(two independent dependency chains keep PE/ACT/DVE busy). Emission is
phased (pre-tanh ops for both halves, then post-tanh ops) to avoid
head-of-line blocking in the engine queues.

hx [67,HC] bf16: rows 0:32 h_{t-1}, 32:64 F_{t-1}, 64 tau, 65 kill, 66 one.
lhs G/P duplicate the h-weights on the F rows, so matmuls see
h_t = h_{t-1}+F_{t-1} without materializing it (hadd runs off-path).

Per half h, per step t:
  PE   MM_G -> bank[0:96,0:HC] = [zc_neg+kill | r | h_n]
  PE   MM_I -> bank[0:32,HC:]  = i_n (start, accumulated)
  PE   MM_P -> bank[32:64,HC:] = hazard pre
  ACT  sigma(bank[0:64,0:HC]) -> SR=[q;r] bf16   (q=(1-z)*m via kill row)
  DVE  hadd: hx_next[0:32] = hx[0:32]+hx[32:64]  (= h_t, off-path)
  DVE  rh = r*h_n (mixed psum read) -> bf16
  PE   MM_ACC: identity@rh accumulated onto i_n -> T2 (stop)
  ACT  tanh(bank[0:64,HC:]) -> Acoll rows 64*(t%2) = [n;a] bf16
  DVE  D = n - h_t ; F = q*D -> hx_next[32:64] ; sq = a*a -> SQcoll
  PE   raw-dot per 2 steps / s-dot per 4 steps (phase-accumulated K=128
       dots into [32,HC] blocks; row order works out to row = t%128),
       emitted one step late to avoid PE head-of-line blocking.
Every 128 steps rawbank/sbank drain via ACT copy to SBUF f32 slabs; the
batched loglik tail runs at the end (2 act-table switches total), masked
accumulation via scalar_tensor_tensor accum_out. Host sums [128,2] f64.
"""

import numpy as np

B, L, H, HH = 4096, 512, 32, 32
EPS = 1e-8
NCORES = 8
BC = B // NCORES       # 512 samples per core
NCH = 3                # phase-shifted chains per core
CW = [172, 170, 170]   # columns per chain
CO = [0, 172, 342]     # column offsets
KILL = -40.0

_CACHE = {}


def _build_module():
    import concourse.bacc as bacc
    import concourse.mybir as mybir
    import concourse.tile as tile

    f32 = mybir.dt.float32
    bf16 = mybir.dt.bfloat16
    AF = mybir.ActivationFunctionType
    ALU = mybir.AluOpType

    nc = bacc.Bacc()

    # xr rows: 0 tau, 1 kill(-40*(1-m)), 2 ones   (bf16, step-major)
    xr_d = nc.dram_tensor("xr", [L, 3, BC], bf16, kind="ExternalInput")
    # lhsg cols: [G(96) | I(32) | P(32) | ident(32)]
    lhsg_d = nc.dram_tensor("lhsg", [67, 192], bf16, kind="ExternalInput")
    # lhsd: 32 R-phase blocks [64,32] then 8 S-phase blocks [128,32]
    lhsd_d = nc.dram_tensor("lhsd", [128, 1280], bf16, kind="ExternalInput")
    cst_d = nc.dram_tensor("cst", [128, 3], f32, kind="ExternalInput")
    hx0_d = nc.dram_tensor("hx0", [67, BC], bf16, kind="ExternalInput")
    # mask in tail layout: [step%128, round, col]
    mt_d = nc.dram_tensor("mt", [128, 4, BC], f32, kind="ExternalInput")
    acc_d = nc.dram_tensor("acc_out", [128, NCH], f32, kind="ExternalOutput")

    NR = 4  # drain rounds per half (512 steps / 128)

    with tile.TileContext(nc) as tc:
        with (
            tc.tile_pool(name="consts", bufs=1) as consts,
            tc.tile_pool(name="hx", bufs=3) as hx_pool,
            tc.tile_pool(name="work", bufs=3) as work,
            tc.tile_pool(name="coll", bufs=2) as coll,
            tc.tile_pool(name="slab", bufs=1) as slabp,
            tc.tile_pool(name="tail", bufs=2) as tailp,
            tc.tile_pool(name="gP", bufs=2, space="PSUM") as gP,
            tc.tile_pool(name="rP", bufs=1, space="PSUM") as rP,
            tc.tile_pool(name="sP", bufs=1, space="PSUM") as sP,
        ):
            lhsg = consts.tile([67, 192], bf16)
            lhsd = consts.tile([128, 1280], bf16)
            cst = consts.tile([128, 3], f32)
            nc.sync.dma_start(lhsg[:], lhsg_d[:])
            nc.sync.dma_start(lhsd[:], lhsd_d[:])
            nc.sync.dma_start(cst[:], cst_d[:])
            lhsG, lhsI = lhsg[:, 0:96], lhsg[:, 96:128]
            lhsP, lhsE = lhsg[:, 128:160], lhsg[0:32, 160:192]
            lhsRp = [lhsd[0:64, 32 * j : 32 * j + 32] for j in range(32)]
            lhsSp = [
                lhsd[:, 1024 + 32 * j : 1024 + 32 * j + 32] for j in range(8)
            ]

            # raw/s slabs: [128, NR, W] f32 per chain
            RAW = [
                slabp.tile([128, NR, CW[h]], f32, name=f"RAW{h}")
                for h in range(NCH)
            ]
            SV = [
                slabp.tile([128, NR, CW[h]], f32, name=f"SV{h}")
                for h in range(NCH)
            ]
            ACC = slabp.tile([128, NCH], f32, name="ACCS")

            hx = [None] * NCH
            for h in range(NCH):
                t0 = hx_pool.tile(
                    [67, CW[h]], bf16, tag=f"hx{h}", name=f"hx0_{h}"
                )
                nc.sync.dma_start(t0[:], hx0_d[:, CO[h] : CO[h] + CW[h]])
                hx[h] = t0

            st = [
                dict(A=None, SQ=None, SR=None, bank=None, hxn=None, Fc=None,
                     dots=[])
                for _ in range(NCH)
            ]
            for h in range(NCH):
                Fc0 = work.tile(
                    [32, CW[h]], bf16, tag=f"Fc{h}", name=f"Fc0_{h}"
                )
                nc.vector.memset(Fc0[:], 0.0)
                st[h]["Fc"] = Fc0
            rawb = sb = None

            for t in range(L):
                if t % 128 == 0:
                    rawb = rP.tile([128, BC], f32, tag="rawb", name=f"rawb{t}")
                    sb = sP.tile([128, BC], f32, tag="sb", name=f"sb{t}")
                last = t == L - 1

                # ---- phase PRE: matmuls + sigma + hadd ----
                for h in range(NCH):
                    s = st[h]
                    W = CW[h]
                    s["A"] = coll.tile(
                        [64, W], bf16, tag=f"A{h}", name=f"A{h}_{t}"
                    )
                    if t % 4 == 0:
                        s["SQ"] = coll.tile(
                            [128, W], bf16, tag=f"SQ{h}", name=f"SQ{h}_{t}"
                        )
                    bank = gP.tile(
                        [128, 2 * W], f32, tag=f"G{h}", name=f"G{h}_{t}"
                    )
                    s["bank"] = bank
                    nc.tensor.matmul(
                        bank[0:96, 0:W], lhsG, hx[h][:], start=True, stop=True
                    )
                    nc.tensor.matmul(
                        bank[0:32, W : 2 * W], lhsI, hx[h][:],
                        start=True, stop=False, skip_group_check=True,
                    )
                    nc.tensor.matmul(
                        bank[32:64, W : 2 * W], lhsP, hx[h][:],
                        start=True, stop=True, tile_position=(0, 32),
                        skip_group_check=True,
                    )
                    SR = work.tile([64, W], bf16, tag=f"SR{h}")
                    s["SR"] = SR
                    nc.scalar.activation(SR[:], bank[0:64, 0:W], AF.Sigmoid)
                    if not last:
                        hxn = hx_pool.tile([67, W], bf16, tag=f"hx{h}")
                        s["hxn"] = hxn
                        nc.gpsimd.tensor_add(
                            hxn[0:32, :], hx[h][0:32, :], s["Fc"]
                        )
                        nc.sync.dma_start(
                            hxn[64:67, :],
                            xr_d[t + 1, :, CO[h] : CO[h] + CW[h]],
                        )

                # rh + T2 accumulation
                for h in range(NCH):
                    s = st[h]
                    W = CW[h]
                    rh = work.tile([32, W], bf16, tag=f"rh{h}")
                    nc.vector.tensor_mul(
                        rh[:], s["SR"][32:64, :], s["bank"][64:96, 0:W]
                    )
                    nc.tensor.matmul(
                        s["bank"][0:32, W : 2 * W], lhsE, rh[:],
                        start=False, stop=True, skip_group_check=True,
                    )

                # deferred dots from the previous step (PE queue: behind MMs)
                for h in range(NCH):
                    for args in st[h]["dots"]:
                        nc.tensor.matmul(*args[0], **args[1])
                    st[h]["dots"] = []

                # ---- phase POST: tanh + h-update + collections ----
                for h in range(NCH):
                    s = st[h]
                    W = CW[h]
                    nc.scalar.activation(
                        s["A"][:], s["bank"][0:64, W : 2 * W], AF.Tanh
                    )

                for h in range(NCH):
                    s = st[h]
                    W = CW[h]
                    if not last:
                        D = work.tile([32, W], bf16, tag=f"D{h}")
                        nc.vector.tensor_sub(
                            D[:], s["A"][0:32, :], s["hxn"][0:32, :]
                        )
                        nc.vector.tensor_mul(
                            s["hxn"][32:64, :], s["SR"][0:32, :], D[:]
                        )
                        Fc = work.tile([32, W], bf16, tag=f"Fc{h}")
                        nc.gpsimd.tensor_copy(Fc[:], s["hxn"][32:64, :])
                        s["Fc"] = Fc
                        hx[h] = s["hxn"]

                for h in range(NCH):
                    s = st[h]
                    W = CW[h]
                    cs = slice(CO[h], CO[h] + W)
                    A, SQ = s["A"], s["SQ"]
                    sr = 32 * (t % 4)
                    nc.vector.tensor_mul(
                        SQ[sr : sr + 32, :], A[32:64, :], A[32:64, :]
                    )

                    tr = t % 128
                    if True:
                        blk, ph = 32 * (tr // 32), tr % 32
                        s["dots"].append((
                            (rawb[blk : blk + 32, cs], lhsRp[ph], A[:]),
                            dict(start=(ph == 0), stop=(ph == 31),
                                 tile_position=(0, blk),
                                 skip_group_check=True),
                        ))
                    if t % 4 == 3:
                        blk, ph = 32 * (tr // 32), (tr % 32) // 4
                        s["dots"].append((
                            (sb[blk : blk + 32, cs], lhsSp[ph], SQ[:]),
                            dict(start=(ph == 0), stop=(ph == 7),
                                 tile_position=(0, blk),
                                 skip_group_check=True),
                        ))
                    if t % 128 == 127 or last:
                        # flush dots before draining on the last round
                        for args in s["dots"]:
                            nc.tensor.matmul(*args[0], **args[1])
                        s["dots"] = []
                    if t % 128 == 127:
                        rnd = t // 128
                        nc.scalar.activation(
                            RAW[h][:, rnd, :], rawb[:, cs], AF.Copy
                        )
                        nc.scalar.activation(
                            SV[h][:, rnd, :], sb[:, cs], AF.Copy
                        )

            # ---- batched loglik tail ----
            b2ap, c0ap, epap = cst[:, 0:1], cst[:, 1:2], cst[:, 2:3]
            MT, SG, ND = [None] * NCH, [None] * NCH, [None] * NCH
            for h in range(NCH):
                W = CW[h]
                MT[h] = tailp.tile([128, NR, W], f32, tag="MT", name=f"MT{h}")
                nc.sync.dma_start(
                    MT[h][:], mt_d[:, :, CO[h] : CO[h] + W]
                )
                SG[h] = tailp.tile([128, NR, W], f32, tag="SG", name=f"SG{h}")
                nc.scalar.activation(
                    SG[h][:], RAW[h][:], AF.Sigmoid, bias=b2ap
                )
                ND[h] = tailp.tile([128, NR, W], f32, tag="ND", name=f"ND{h}")
                nc.vector.scalar_tensor_tensor(
                    ND[h][:], SV[h][:], c0ap, SG[h][:],
                    op0=ALU.subtract, op1=ALU.mult,
                )
            for h in range(NCH):
                W = CW[h]
                EX = tailp.tile([128, NR, W], f32, tag="EX", name=f"EX{h}")
                nc.scalar.activation(EX[:], RAW[h][:], AF.Exp, bias=b2ap)
                PH = tailp.tile([128, NR, W], f32, tag="PH", name=f"PH{h}")
                nc.scalar.activation(PH[:], EX[:], AF.Ln, bias=1.0)
                EX2 = tailp.tile([128, NR, W], f32, tag="EX2", name=f"EX2{h}")
                nc.scalar.activation(EX2[:], ND[h][:], AF.Exp, scale=-1.0)
                SPD = tailp.tile([128, NR, W], f32, tag="SPD", name=f"SPD{h}")
                nc.scalar.activation(SPD[:], EX2[:], AF.Ln, bias=1.0)
                LGL = tailp.tile([128, NR, W], f32, tag="LGL", name=f"LGL{h}")
                nc.scalar.activation(LGL[:], SPD[:], AF.Ln, bias=epap)
                LLd = tailp.tile([128, NR, W], f32, tag="LL", name=f"LL{h}")
                nc.vector.tensor_sub(LLd[:], LGL[:], PH[:])
                LLM = tailp.tile([128, NR, W], f32, tag="LLM", name=f"LLM{h}")
                nc.vector.scalar_tensor_tensor(
                    LLM[:], LLd[:], 0.0, MT[h][:],
                    op0=ALU.add, op1=ALU.mult,
                    accum_out=ACC[:, h : h + 1],
                )
            nc.sync.dma_start(acc_d[:], ACC[:])

    nc.finalize()
    return nc


def _prep_host(inputs):
    import ml_dtypes

    bf = ml_dtypes.bfloat16
    d = {k: np.asarray(v, np.float32) for k, v in inputs.items()}
    w_ih, w_hh = d["w_ih"], d["w_hh"]
    b_ih, b_hh = d["b_ih"], d["b_hh"]
    w1, b1, w2, b2 = d["w1"], d["b1"], d["w2"], d["b2"]
    w1_tau, w1_h = w1[:, 0], w1[:, 1:]

    # G blocks: [zc_neg(0:32) | r(32:64) | h_n(64:96)], h-weights
    # duplicated on F rows 32:64
    lhsG = np.zeros((67, 96), np.float32)
    hw = np.zeros((32, 96), np.float32)
    hw[:, 0:32] = -w_hh[32:64, :].T
    hw[:, 32:64] = w_hh[0:32, :].T
    hw[:, 64:96] = w_hh[64:96, :].T
    lhsG[0:32] = hw
    lhsG[32:64] = hw
    lhsG[64, 0:32] = -w_ih[32:64, 0]
    lhsG[64, 32:64] = w_ih[0:32, 0]
    lhsG[65, 0:32] = 1.0                       # kill row -> zc block
    lhsG[66, 0:32] = -(b_ih[32:64] + b_hh[32:64])
    lhsG[66, 32:64] = b_ih[0:32] + b_hh[0:32]
    lhsG[66, 64:96] = b_hh[64:96]

    lhsI = np.zeros((67, 32), np.float32)
    lhsI[64, :] = w_ih[64:96, 0]
    lhsI[66, :] = b_ih[64:96]

    lhsP = np.zeros((67, 32), np.float32)
    lhsP[0:32, :] = w1_h.T
    lhsP[32:64, :] = w1_h.T
    lhsP[64, :] = w1_tau
    lhsP[66, :] = b1

    lhsg = np.zeros((67, 192), np.float32)
    lhsg[:, 0:96] = lhsG
    lhsg[:, 96:128] = lhsI
    lhsg[:, 128:160] = lhsP
    lhsg[0:32, 160:192] = np.eye(32, dtype=np.float32)

    c = w1_tau * w2
    lhsd = np.zeros((128, 1280), np.float32)
    for j in range(32):  # raw-dot phase j: out row j, K=64, a at rows 32:64
        lhsd[32:64, 32 * j + j] = w2
    for j in range(8):
        for g in range(4):
            lhsd[32 * g : 32 * g + 32, 1024 + 32 * j + 4 * j + g] = c

    cst = np.zeros((128, 3), np.float32)
    cst[:, 0] = b2[0]
    cst[:, 1] = c.sum()
    cst[:, 2] = EPS

    deltas, mask = d["deltas"], d["mask"]
    in_maps = []
    for i in range(NCORES):
        sl = slice(i * BC, (i + 1) * BC)
        dT = deltas[sl].T          # [L, BC]
        mT = mask[sl].T            # [L, BC]
        xr = np.empty((L, 3, BC), np.float32)
        xr[:, 0, :] = dT
        xr[:, 1, :] = KILL * (1.0 - mT)
        xr[:, 2, :] = 1.0
        hx0 = np.zeros((67, BC), np.float32)
        hx0[64:67, :] = xr[0]
        mt = np.empty((128, 4, BC), np.float32)
        for r in range(4):
            mt[:, r, :] = mT[128 * r : 128 * (r + 1), :]
        in_maps.append(
            {
                "xr": xr.astype(bf),
                "lhsg": lhsg.astype(bf),
                "lhsd": lhsd.astype(bf),
                "cst": cst,
                "hx0": hx0.astype(bf),
                "mt": mt,
            }
        )
    return in_maps


def run_on_device(inputs, trace=False):
    from concourse.bass_utils import run_bass_kernel_spmd

    if "nc" not in _CACHE:
        _CACHE["nc"] = _build_module()
    nc = _CACHE["nc"]
    in_maps = _prep_host(inputs)
    res = run_bass_kernel_spmd(nc, in_maps, core_ids=list(range(NCORES)), trace=trace)
    tot = 0.0
    for r in res.results:
        tot += np.asarray(r["acc_out"], np.float64).sum()
    msum = np.asarray(inputs["mask"], np.float64).sum()
    out = np.float32(tot / (msum + EPS))
    return np.asarray(out, np.float32), res


def kernel(**inputs):
    out, _ = run_on_device(inputs, trace=False)
    return out
